# revision 17
# baseline (speedup 1.0000x reference)
"""ConvNeXt block (nn_CNBlock) Trainium2 Bass kernel.

Reference computation (per image, fp32):
  y = depthwise_conv7x7(x, conv_w) + conv_b          # NCHW, pad 3
  y = LayerNorm_channel(y) * ln_g + ln_b             # over C at each pixel
  h = gelu(y^T @ w1 + b1, exact)                     # C -> 4C
  out = h @ w2 + b2                                  # 4C -> C  (NCHW out)

Strategy: data-parallel over batch, 4 images per NeuronCore x 8 cores.
Per core, channels-first layout [C=2x128 partitions, pixels]:
  - conv: 32 taps on PE as diagonal-matrix matmuls (f32r) accumulated in
    PSUM + 17 taps as DVE fused scalar_tensor_tensor MACs (fp32).
  - LN: ones-matrix matmuls give per-pixel sums broadcast across all 128
    partitions in PSUM; variance/rsqrt via DVE/ACT; normalize on DVE.
    (ln affine folded into w1/b1 on host.)
  - MLP: f32r matmuls on PE, exact-erf Gelu + biases on ACT.
All matmul operands are float32r (TF32-like, ~1.5e-4 rel err, 4x faster
than fp32 on the PE).
"""
import sys

sys.path.insert(0, "/opt/trn_rl_repo")

import numpy as np

# ---------------- problem constants (hardcoded) ----------------
B, DIM, H, W = 32, 256, 56, 56
HID = 4 * DIM
EPS = 1e-6
NCORES = 8
BS = B // NCORES          # images per core
NCT = 2                   # channel tiles of 128
G = W + 6                 # padded grid width 62
NP = H * G                # conv output positions incl. garbage cols = 3472
XPL = 3856                # padded input tile length (3 + 62*62 + slack)
NCH = 8                   # pixel chunks
CW = NP // NCH            # chunk width 434 (= 7 rows of 62)
OW = 7 * W                # valid outputs per chunk 392
N_PE_TAPS = 32
N_DVE_TAPS = 49 - N_PE_TAPS

_CACHE = {}


def _taps():
    # (off, di, dj) for all 49 taps; off is the flat shift in the padded grid
    taps = []
    for di in range(7):
        for dj in range(7):
            taps.append((di * G + dj - 3, di, dj))
    return taps[:N_PE_TAPS], taps[N_PE_TAPS:]


def _build_program():
    import concourse.bacc as bacc
    import concourse.mybir as mybir
    import concourse.tile as tile

    dt = mybir.dt
    AF = mybir.ActivationFunctionType
    ALU = mybir.AluOpType
    F32R = dt.float32r
    F32 = dt.float32

    pe_taps, dve_taps = _taps()

    nc = bacc.Bacc("TRN2", target_bir_lowering=False, debug=False)

    d_xpad = nc.dram_tensor("xpad", [BS, NCT, 128, XPL], F32R, kind="ExternalInput")
    d_w1 = nc.dram_tensor("w1sb", [128, 2048], F32R, kind="ExternalInput")
    d_w2 = nc.dram_tensor("w2sb", [128, 2048], F32R, kind="ExternalInput")
    d_dg = nc.dram_tensor("dgsb", [128, NCT * N_PE_TAPS * 128], F32R, kind="ExternalInput")
    d_ones = nc.dram_tensor("ones128", [128, 128], F32R, kind="ExternalInput")
    # fp32 const columns: 0-1 cb, 2-9 b1eff, 10-11 b2, 12 eps, 13.. dve tap weights
    NC_CONST = 13 + NCT * N_DVE_TAPS
    d_cst = nc.dram_tensor("cstf", [128, NC_CONST], F32, kind="ExternalInput")
    d_out = nc.dram_tensor("yout", [BS, NCT, 128, H * W], F32, kind="ExternalOutput")

    with tile.TileContext(nc) as tc:
        with (
            tc.tile_pool(name="static", bufs=1) as stat,
            tc.tile_pool(name="xp", bufs=2) as p_xp,
            tc.tile_pool(name="yacc", bufs=3) as p_yacc,
            tc.tile_pool(name="y2", bufs=1) as p_y2,
            tc.tile_pool(name="yhat", bufs=2) as p_yhat,
            tc.tile_pool(name="hb", bufs=1) as p_h,
            tc.tile_pool(name="outc", bufs=2) as p_out,
            tc.tile_pool(name="var", bufs=1) as p_var,
            tc.tile_pool(name="ydve", bufs=1) as p_ydve,
            tc.tile_pool(name="pconv", bufs=2, space="PSUM") as ps_conv,
            tc.tile_pool(name="psy", bufs=1, space="PSUM") as ps_sy,
            tc.tile_pool(name="psy2", bufs=1, space="PSUM") as ps_sy2,
            tc.tile_pool(name="ph", bufs=2, space="PSUM") as ps_h,
            tc.tile_pool(name="po", bufs=2, space="PSUM") as ps_o,
        ):
            w1sb = stat.tile([128, 2048], F32R, name="w1sb")
            w2sb = stat.tile([128, 2048], F32R, name="w2sb")
            dgsb = stat.tile([128, NCT * N_PE_TAPS * 128], F32R, name="dgsb")
            ones128 = stat.tile([128, 128], F32R, name="ones128")
            cst = stat.tile([128, NC_CONST], F32, name="cst")
            nc.sync.dma_start(dgsb[:], d_dg.ap())
            nc.sync.dma_start(cst[:], d_cst.ap())

            yaccs = {}
            vars_ = {}
            xps = {}

            def conv_dve(b):
                for ct in range(NCT):
                    xp = xps[(b, ct)]
                    ya = yaccs[(b, ct)]
                    yd = p_ydve.tile([128, NP], F32, name=f"yd_{b}_{ct}", tag="yd")
                    yd_v = yd[:, 0:NP].rearrange("p (r g) -> p r g", g=G)[:, :, 3:59]
                    for j, (off, _, _) in enumerate(dve_taps):
                        xv = xp[:, 6 + off: 6 + off + NP].rearrange(
                            "p (r g) -> p r g", g=G
                        )[:, :, 0:56]
                        wcol = cst[:, 13 + ct * N_DVE_TAPS + j: 14 + ct * N_DVE_TAPS + j]
                        if j == 0:
                            nc.vector.tensor_scalar(
                                yd_v, xv.bitcast(F32), wcol, None, op0=ALU.mult,
                            )
                        else:
                            nc.vector.scalar_tensor_tensor(
                                yd_v, xv.bitcast(F32), wcol, yd_v,
                                op0=ALU.mult, op1=ALU.add,
                            )
                    # merge DVE partial into ya on GpSimd (valid cols only),
                    # split in half to limit head-of-line blocking
                    ya_v = ya[:, 0:NP].rearrange("p (r g) -> p r g", g=G)[:, :, 3:59]
                    half = 28
                    nc.gpsimd.tensor_tensor(
                        ya_v[:, :half], ya_v[:, :half].bitcast(F32),
                        yd_v[:, :half], op=ALU.add)
                    nc.gpsimd.tensor_tensor(
                        ya_v[:, half:], ya_v[:, half:].bitcast(F32),
                        yd_v[:, half:], op=ALU.add)

            def conv_pe(b):
                for ct in range(NCT):
                    xp = p_xp.tile([128, XPL], F32R, name=f"xp_{b}_{ct}", tag="xp")
                    nc.sync.dma_start(xp[:], d_xpad.ap()[b, ct])
                    xps[(b, ct)] = xp
                    ya = p_yacc.tile([128, NP], F32R, name=f"ya_{b}_{ct}", tag="yacc")
                    yaccs[(b, ct)] = ya
                    for ch in range(NCH):
                        pc = ps_conv.tile([128, CW], F32, name=f"pc_{b}_{ct}_{ch}", tag="pc")
                        q0 = ch * CW
                        for i, (off, _, _) in enumerate(pe_taps):
                            nc.tensor.matmul(
                                pc[:],
                                dgsb[:, (ct * N_PE_TAPS + i) * 128:(ct * N_PE_TAPS + i + 1) * 128],
                                xp[:, 3 + off + q0: 3 + off + q0 + CW],
                                start=(i == 0),
                                stop=(i == N_PE_TAPS - 1),
                            )
                        nc.scalar.activation(
                            ya[:, q0:q0 + CW], pc[:], AF.Identity,
                            bias=cst[:, ct:ct + 1],
                        )

            def stats_phase(b):
                ya0 = yaccs[(b, 0)]
                ya1 = yaccs[(b, 1)]
                va = p_var.tile([128, NP], F32, name=f"va_{b}", tag="va")
                vars_[b] = va
                for ch in range(NCH):
                    q0 = ch * CW
                    sl = slice(q0, q0 + CW)
                    y2 = p_y2.tile([128, 2 * CW], F32R, name=f"y2_{b}_{ch}", tag="y2")
                    nc.scalar.activation(y2[:, 0:CW], ya0[:, sl].bitcast(F32), AF.Square)
                    nc.scalar.activation(y2[:, CW:], ya1[:, sl].bitcast(F32), AF.Square)
                    psy = ps_sy.tile([128, CW], F32, name=f"psy_{b}_{ch}", tag="psy")
                    nc.tensor.matmul(psy[:], ones128[:], ya0[:, sl], start=True, stop=False)
                    nc.tensor.matmul(psy[:], ones128[:], ya1[:, sl], start=False, stop=True)
                    psy2 = ps_sy2.tile([128, CW], F32, name=f"psy2_{b}_{ch}", tag="psy2")
                    nc.tensor.matmul(psy2[:], ones128[:], y2[:, 0:CW], start=True, stop=False)
                    nc.tensor.matmul(psy2[:], ones128[:], y2[:, CW:], start=False, stop=True)
                    # var slice: mu, mu^2, then sy2/256 - mu^2
                    vsl = va[:, sl]
                    nc.scalar.activation(vsl, psy[:], AF.Copy, bias=0.0, scale=1.0 / DIM)
                    nc.scalar.activation(vsl, vsl, AF.Square)
                    nc.vector.scalar_tensor_tensor(
                        vsl, psy2[:], 1.0 / DIM, vsl, op0=ALU.mult, op1=ALU.subtract
                    )
                    # center y in place: y -= mu
                    for ya in (ya0, ya1):
                        nc.vector.scalar_tensor_tensor(
                            ya[:, sl], psy[:], -1.0 / DIM, ya[:, sl].bitcast(F32),
                            op0=ALU.mult, op1=ALU.add,
                        )
                # batched rsqrt: r = 1/sqrt(var + eps), in place, one table switch
                nc.scalar.activation(va[:], va[:], AF.Abs_reciprocal_sqrt, bias=cst[:, 12:13])

            def mlp_phase(b):
                ya0 = yaccs[(b, 0)]
                ya1 = yaccs[(b, 1)]
                va = vars_[b]
                for ch in range(NCH):
                    q0 = ch * CW
                    sl = slice(q0, q0 + CW)
                    yh = p_yhat.tile([128, 2 * CW], F32R, name=f"yh_{b}_{ch}", tag="yh")
                    for ct, ya in ((0, ya0), (1, ya1)):
                        nc.gpsimd.tensor_tensor(
                            yh[:, ct * CW:(ct + 1) * CW], ya[:, sl].bitcast(F32),
                            va[:, sl], op=ALU.mult,
                        )
                    hb = p_h.tile([128, 8 * CW], F32R, name=f"hb_{b}_{ch}", tag="hb")
                    for f in range(8):
                        ph = ps_h.tile([128, CW], F32, name=f"ph_{b}_{ch}_{f}", tag="ph")
                        nc.tensor.matmul(
                            ph[:], w1sb[:, f * 128:(f + 1) * 128], yh[:, 0:CW],
                            start=True, stop=False,
                        )
                        nc.tensor.matmul(
                            ph[:], w1sb[:, 1024 + f * 128:1024 + (f + 1) * 128],
                            yh[:, CW:], start=False, stop=True,
                        )
                        nc.scalar.activation(
                            hb[:, f * CW:(f + 1) * CW], ph[:], AF.Gelu,
                            bias=cst[:, 2 + f:3 + f],
                        )
                    oc = p_out.tile([128, 2 * CW], F32, name=f"oc_{b}_{ch}", tag="oc")
                    for ct in range(NCT):
                        po = ps_o.tile([128, CW], F32, name=f"po_{b}_{ch}_{ct}", tag="po")
                        for f in range(8):
                            nc.tensor.matmul(
                                po[:], w2sb[:, f * 256 + ct * 128: f * 256 + (ct + 1) * 128],
                                hb[:, f * CW:(f + 1) * CW],
                                start=(f == 0), stop=(f == 7),
                            )
                        nc.scalar.activation(
                            oc[:, ct * CW:(ct + 1) * CW], po[:], AF.Identity,
                            bias=cst[:, 10 + ct:11 + ct],
                        )
                        src = oc[:, ct * CW:(ct + 1) * CW].rearrange(
                            "p (r w) -> p r w", r=7
                        )[:, :, 3:59]
                        dst = d_out.ap()[b, ct, :, ch * OW:(ch + 1) * OW].rearrange(
                            "p (r w) -> p r w", w=W
                        )
                        nc.scalar.dma_start(dst, src)

            # software pipeline: conv one image ahead; stats DVE ops are
            # emitted before the next image's DVE tap chains
            conv_pe(0); conv_dve(0)
            nc.sync.dma_start(w1sb[:], d_w1.ap())
            nc.sync.dma_start(w2sb[:], d_w2.ap())
            nc.sync.dma_start(ones128[:], d_ones.ap())
            conv_pe(1)
            stats_phase(0)
            conv_dve(1)
            for b in range(BS):
                mlp_phase(b)
                if b + 1 < BS:
                    stats_phase(b + 1)
                if b + 2 < BS:
                    conv_pe(b + 2)
                    conv_dve(b + 2)

    nc.compile()
    return nc


def _host_prep(x, conv_w, conv_b, ln_g, ln_b, w1, b1, w2, b2):
    """Returns (shared static arrays dict, per-core xpad list)."""
    f32 = np.float32
    x = np.asarray(x, f32)
    conv_w = np.asarray(conv_w, f32)
    conv_b = np.asarray(conv_b, f32)
    ln_g = np.asarray(ln_g, f32)
    ln_b = np.asarray(ln_b, f32)
    w1 = np.asarray(w1, f32)
    b1 = np.asarray(b1, f32)
    w2 = np.asarray(w2, f32)
    b2 = np.asarray(b2, f32)

    pe_taps, dve_taps = _taps()

    # fold LN affine into w1/b1
    w1g = (ln_g[:, None] * w1).astype(f32)                  # [256, 1024]
    b1e = (ln_b @ w1 + b1).astype(f32)                      # [1024]

    # w1sb[c, ct*1024 + f*128 + j] = w1g[ct*128 + c, f*128 + j]
    w1sb = np.ascontiguousarray(
        w1g.reshape(2, 128, 8, 128).transpose(1, 0, 2, 3).reshape(128, 2048)
    )
    # w2sb[h, f*256 + ct*128 + co] = w2[f*128 + h, ct*128 + co]
    w2sb = np.ascontiguousarray(
        w2.reshape(8, 128, 2, 128).transpose(1, 0, 2, 3).reshape(128, 2048)
    )
    # diagonal conv matrices for PE taps
    dgsb = np.zeros((128, NCT * N_PE_TAPS * 128), f32)
    idx = np.arange(128)
    for ct in range(NCT):
        for i, (_, di, dj) in enumerate(pe_taps):
            dgsb[idx, (ct * N_PE_TAPS + i) * 128 + idx] = conv_w[ct * 128 + idx, 0, di, dj]
    ones128 = np.ones((128, 128), f32)

    NC_CONST = 13 + NCT * N_DVE_TAPS
    cst = np.zeros((128, NC_CONST), f32)
    cst[:, 0] = conv_b[:128]
    cst[:, 1] = conv_b[128:]
    cst[:, 2:10] = b1e.reshape(8, 128).T
    cst[:, 10] = b2[:128]
    cst[:, 11] = b2[128:]
    cst[:, 12] = EPS
    for ct in range(NCT):
        for j, (_, di, dj) in enumerate(dve_taps):
            cst[:, 13 + ct * N_DVE_TAPS + j] = conv_w[ct * 128 + idx, 0, di, dj]

    # padded input grids
    xg = np.zeros((B, DIM, G, G), f32)
    xg[:, :, 3:59, 3:59] = x
    xg = xg.reshape(B, NCT, 128, G * G)
    xpad = np.zeros((B, NCT, 128, XPL), f32)
    xpad[:, :, :, 3:3 + G * G] = xg

    static = dict(w1sb=w1sb, w2sb=w2sb, dgsb=dgsb, ones128=ones128, cstf=cst)
    xpads = [np.ascontiguousarray(xpad[c * BS:(c + 1) * BS]) for c in range(NCORES)]
    return static, xpads


def kernel(**inputs) -> np.ndarray:
    from concourse import bass_utils

    if "nc" not in _CACHE:
        _CACHE["nc"] = _build_program()
    nc = _CACHE["nc"]

    static, xpads = _host_prep(**inputs)
    in_maps = [dict(static, xpad=xpads[c]) for c in range(NCORES)]
    res = bass_utils.run_bass_kernel_spmd(nc, in_maps, core_ids=list(range(NCORES)))

    out = np.empty((B, DIM, H, W), np.float32)
    for c in range(NCORES):
        yo = res.results[c]["yout"].reshape(BS, NCT, 128, H, W)
        for b in range(BS):
            out[c * BS + b, :128] = yo[b, 0]
            out[c * BS + b, 128:] = yo[b, 1]
    return out


# revision 18
# speedup vs baseline: 1.0341x; 1.0341x over previous
"""ConvNeXt block (nn_CNBlock) Trainium2 Bass kernel.

Reference computation (per image, fp32):
  y = depthwise_conv7x7(x, conv_w) + conv_b          # NCHW, pad 3
  y = LayerNorm_channel(y) * ln_g + ln_b             # over C at each pixel
  h = gelu(y^T @ w1 + b1, exact)                     # C -> 4C
  out = h @ w2 + b2                                  # 4C -> C  (NCHW out)

Strategy: data-parallel over batch, 4 images per NeuronCore x 8 cores.
Per core, channels-first layout [C=2x128 partitions, pixels]:
  - conv: 32 taps on PE as diagonal-matrix matmuls (f32r) accumulated in
    PSUM + 17 taps as DVE fused scalar_tensor_tensor MACs (fp32).
  - LN: ones-matrix matmuls give per-pixel sums broadcast across all 128
    partitions in PSUM; variance/rsqrt via DVE/ACT; normalize on DVE.
    (ln affine folded into w1/b1 on host.)
  - MLP: f32r matmuls on PE, exact-erf Gelu + biases on ACT.
All matmul operands are float32r (TF32-like, ~1.5e-4 rel err, 4x faster
than fp32 on the PE).
"""
import sys

sys.path.insert(0, "/opt/trn_rl_repo")

import numpy as np

# ---------------- problem constants (hardcoded) ----------------
B, DIM, H, W = 32, 256, 56, 56
HID = 4 * DIM
EPS = 1e-6
NCORES = 8
BS = B // NCORES          # images per core
NCT = 2                   # channel tiles of 128
G = W + 6                 # padded grid width 62
NP = H * G                # conv output positions incl. garbage cols = 3472
XPL = 3856                # padded input tile length (3 + 62*62 + slack)
NCH = 8                   # pixel chunks
CW = NP // NCH            # chunk width 434 (= 7 rows of 62)
OW = 7 * W                # valid outputs per chunk 392
N_PE_TAPS = 32
N_DVE_TAPS = 49 - N_PE_TAPS

_CACHE = {}


def _taps():
    # (off, di, dj) for all 49 taps; off is the flat shift in the padded grid
    taps = []
    for di in range(7):
        for dj in range(7):
            taps.append((di * G + dj - 3, di, dj))
    return taps[:N_PE_TAPS], taps[N_PE_TAPS:]


def _build_program():
    import concourse.bacc as bacc
    import concourse.mybir as mybir
    import concourse.tile as tile

    dt = mybir.dt
    AF = mybir.ActivationFunctionType
    ALU = mybir.AluOpType
    F32R = dt.float32r
    F32 = dt.float32

    pe_taps, dve_taps = _taps()

    nc = bacc.Bacc("TRN2", target_bir_lowering=False, debug=False)

    d_xpad = nc.dram_tensor("xpad", [BS, NCT, 128, XPL], F32R, kind="ExternalInput")
    d_w1 = nc.dram_tensor("w1sb", [128, 2048], F32R, kind="ExternalInput")
    d_w2 = nc.dram_tensor("w2sb", [128, 2048], F32R, kind="ExternalInput")
    d_dg = nc.dram_tensor("dgsb", [128, NCT * N_PE_TAPS * 128], F32R, kind="ExternalInput")
    d_ones = nc.dram_tensor("ones128", [128, 128], F32R, kind="ExternalInput")
    # fp32 const columns: 0-1 cb, 2-9 b1eff, 10-11 b2, 12 eps, 13.. dve tap weights
    NC_CONST = 13 + NCT * N_DVE_TAPS
    d_cst = nc.dram_tensor("cstf", [128, NC_CONST], F32, kind="ExternalInput")
    d_out = nc.dram_tensor("yout", [BS, NCT, 128, H * W], F32, kind="ExternalOutput")

    with tile.TileContext(nc) as tc:
        with (
            tc.tile_pool(name="static", bufs=1) as stat,
            tc.tile_pool(name="xp", bufs=2) as p_xp,
            tc.tile_pool(name="yacc", bufs=3) as p_yacc,
            tc.tile_pool(name="y2", bufs=1) as p_y2,
            tc.tile_pool(name="yhat", bufs=2) as p_yhat,
            tc.tile_pool(name="hb", bufs=1) as p_h,
            tc.tile_pool(name="outc", bufs=2) as p_out,
            tc.tile_pool(name="var", bufs=1) as p_var,
            tc.tile_pool(name="ydve", bufs=1) as p_ydve,
            tc.tile_pool(name="pconv", bufs=2, space="PSUM") as ps_conv,
            tc.tile_pool(name="psy", bufs=1, space="PSUM") as ps_sy,
            tc.tile_pool(name="psy2", bufs=1, space="PSUM") as ps_sy2,
            tc.tile_pool(name="ph", bufs=2, space="PSUM") as ps_h,
            tc.tile_pool(name="po", bufs=2, space="PSUM") as ps_o,
        ):
            w1sb = stat.tile([128, 2048], F32R, name="w1sb")
            w2sb = stat.tile([128, 2048], F32R, name="w2sb")
            dgsb = stat.tile([128, NCT * N_PE_TAPS * 128], F32R, name="dgsb")
            ones128 = stat.tile([128, 128], F32R, name="ones128")
            cst = stat.tile([128, NC_CONST], F32, name="cst")
            nc.sync.dma_start(dgsb[:], d_dg.ap())
            nc.sync.dma_start(cst[:], d_cst.ap())

            yaccs = {}
            vars_ = {}
            xps = {}

            def conv_dve(b):
                for ct in range(NCT):
                    xp = xps[(b, ct)]
                    ya = yaccs[(b, ct)]
                    yd = p_ydve.tile([128, NP], F32, name=f"yd_{b}_{ct}", tag="yd")
                    yd_v = yd[:, 0:NP].rearrange("p (r g) -> p r g", g=G)[:, :, 3:59]
                    for j, (off, _, _) in enumerate(dve_taps):
                        xv = xp[:, 6 + off: 6 + off + NP].rearrange(
                            "p (r g) -> p r g", g=G
                        )[:, :, 0:56]
                        wcol = cst[:, 13 + ct * N_DVE_TAPS + j: 14 + ct * N_DVE_TAPS + j]
                        if j == 0:
                            nc.vector.tensor_scalar(
                                yd_v, xv.bitcast(F32), wcol, None, op0=ALU.mult,
                            )
                        else:
                            nc.vector.scalar_tensor_tensor(
                                yd_v, xv.bitcast(F32), wcol, yd_v,
                                op0=ALU.mult, op1=ALU.add,
                            )
                    # merge DVE partial into ya on GpSimd (valid cols only),
                    # split in half to limit head-of-line blocking
                    ya_v = ya[:, 0:NP].rearrange("p (r g) -> p r g", g=G)[:, :, 3:59]
                    half = 28
                    nc.gpsimd.tensor_tensor(
                        ya_v[:, :half], ya_v[:, :half].bitcast(F32),
                        yd_v[:, :half], op=ALU.add)
                    nc.gpsimd.tensor_tensor(
                        ya_v[:, half:], ya_v[:, half:].bitcast(F32),
                        yd_v[:, half:], op=ALU.add)

            def conv_pe(b):
                for ct in range(NCT):
                    xp = p_xp.tile([128, XPL], F32R, name=f"xp_{b}_{ct}", tag="xp")
                    nc.sync.dma_start(xp[:], d_xpad.ap()[b, ct])
                    xps[(b, ct)] = xp
                    ya = p_yacc.tile([128, NP], F32R, name=f"ya_{b}_{ct}", tag="yacc")
                    yaccs[(b, ct)] = ya
                    for ch in range(NCH):
                        pc = ps_conv.tile([128, CW], F32, name=f"pc_{b}_{ct}_{ch}", tag="pc")
                        q0 = ch * CW
                        for i, (off, _, _) in enumerate(pe_taps):
                            nc.tensor.matmul(
                                pc[:],
                                dgsb[:, (ct * N_PE_TAPS + i) * 128:(ct * N_PE_TAPS + i + 1) * 128],
                                xp[:, 3 + off + q0: 3 + off + q0 + CW],
                                start=(i == 0),
                                stop=(i == N_PE_TAPS - 1),
                            )
                        nc.scalar.activation(
                            ya[:, q0:q0 + CW], pc[:], AF.Identity,
                            bias=cst[:, ct:ct + 1],
                        )

            def stats_phase(b):
                ya0 = yaccs[(b, 0)]
                ya1 = yaccs[(b, 1)]
                va = p_var.tile([128, NP], F32, name=f"va_{b}", tag="va")
                vars_[b] = va
                for ch in range(NCH):
                    q0 = ch * CW
                    sl = slice(q0, q0 + CW)
                    y2 = p_y2.tile([128, 2 * CW], F32R, name=f"y2_{b}_{ch}", tag="y2")
                    nc.scalar.activation(y2[:, 0:CW], ya0[:, sl].bitcast(F32), AF.Square)
                    nc.scalar.activation(y2[:, CW:], ya1[:, sl].bitcast(F32), AF.Square)
                    psy = ps_sy.tile([128, CW], F32, name=f"psy_{b}_{ch}", tag="psy")
                    nc.tensor.matmul(psy[:], ones128[:], ya0[:, sl], start=True, stop=False)
                    nc.tensor.matmul(psy[:], ones128[:], ya1[:, sl], start=False, stop=True)
                    psy2 = ps_sy2.tile([128, CW], F32, name=f"psy2_{b}_{ch}", tag="psy2")
                    nc.tensor.matmul(psy2[:], ones128[:], y2[:, 0:CW], start=True, stop=False)
                    nc.tensor.matmul(psy2[:], ones128[:], y2[:, CW:], start=False, stop=True)
                    # var slice: mu, mu^2, then sy2/256 - mu^2
                    vsl = va[:, sl]
                    nc.scalar.activation(vsl, psy[:], AF.Copy, bias=0.0, scale=1.0 / DIM)
                    nc.scalar.activation(vsl, vsl, AF.Square)
                    nc.vector.scalar_tensor_tensor(
                        vsl, psy2[:], 1.0 / DIM, vsl, op0=ALU.mult, op1=ALU.subtract
                    )
                    # center y in place: y -= mu
                    for ya in (ya0, ya1):
                        nc.vector.scalar_tensor_tensor(
                            ya[:, sl], psy[:], -1.0 / DIM, ya[:, sl].bitcast(F32),
                            op0=ALU.mult, op1=ALU.add,
                        )
                # batched rsqrt: r = 1/sqrt(var + eps), in place, one table switch
                nc.scalar.activation(va[:], va[:], AF.Abs_reciprocal_sqrt, bias=cst[:, 12:13])

            def mlp_phase(b):
                ya0 = yaccs[(b, 0)]
                ya1 = yaccs[(b, 1)]
                va = vars_[b]
                for ch in range(NCH):
                    q0 = ch * CW
                    sl = slice(q0, q0 + CW)
                    yh = p_yhat.tile([128, 2 * CW], F32R, name=f"yh_{b}_{ch}", tag="yh")
                    for ct, ya in ((0, ya0), (1, ya1)):
                        nc.gpsimd.tensor_tensor(
                            yh[:, ct * CW:(ct + 1) * CW], ya[:, sl].bitcast(F32),
                            va[:, sl], op=ALU.mult,
                        )
                    hb = p_h.tile([128, 8 * CW], F32R, name=f"hb_{b}_{ch}", tag="hb")
                    for f in range(8):
                        ph = ps_h.tile([128, CW], F32, name=f"ph_{b}_{ch}_{f}", tag="ph")
                        nc.tensor.matmul(
                            ph[:], w1sb[:, f * 128:(f + 1) * 128], yh[:, 0:CW],
                            start=True, stop=False,
                        )
                        nc.tensor.matmul(
                            ph[:], w1sb[:, 1024 + f * 128:1024 + (f + 1) * 128],
                            yh[:, CW:], start=False, stop=True,
                        )
                        nc.scalar.activation(
                            hb[:, f * CW:(f + 1) * CW], ph[:], AF.Gelu,
                            bias=cst[:, 2 + f:3 + f],
                        )
                    oc = p_out.tile([128, 2 * CW], F32, name=f"oc_{b}_{ch}", tag="oc")
                    for ct in range(NCT):
                        po = ps_o.tile([128, CW], F32, name=f"po_{b}_{ch}_{ct}", tag="po")
                        for f in range(8):
                            nc.tensor.matmul(
                                po[:], w2sb[:, f * 256 + ct * 128: f * 256 + (ct + 1) * 128],
                                hb[:, f * CW:(f + 1) * CW],
                                start=(f == 0), stop=(f == 7),
                            )
                        nc.scalar.activation(
                            oc[:, ct * CW:(ct + 1) * CW], po[:], AF.Identity,
                            bias=cst[:, 10 + ct:11 + ct],
                        )
                        src = oc[:, ct * CW:(ct + 1) * CW].rearrange(
                            "p (r w) -> p r w", r=7
                        )[:, :, 3:59]
                        dst = d_out.ap()[b, ct, :, ch * OW:(ch + 1) * OW].rearrange(
                            "p (r w) -> p r w", w=W
                        )
                        nc.sync.dma_start(dst, src)

            # software pipeline: conv one image ahead; stats DVE ops are
            # emitted before the next image's DVE tap chains
            conv_pe(0); conv_dve(0)
            nc.sync.dma_start(w1sb[:], d_w1.ap())
            nc.sync.dma_start(w2sb[:], d_w2.ap())
            nc.sync.dma_start(ones128[:], d_ones.ap())
            conv_pe(1)
            stats_phase(0)
            conv_dve(1)
            for b in range(BS):
                mlp_phase(b)
                if b + 1 < BS:
                    stats_phase(b + 1)
                if b + 2 < BS:
                    conv_pe(b + 2)
                    conv_dve(b + 2)

    nc.compile()
    return nc


def _host_prep(x, conv_w, conv_b, ln_g, ln_b, w1, b1, w2, b2):
    """Returns (shared static arrays dict, per-core xpad list)."""
    f32 = np.float32
    x = np.asarray(x, f32)
    conv_w = np.asarray(conv_w, f32)
    conv_b = np.asarray(conv_b, f32)
    ln_g = np.asarray(ln_g, f32)
    ln_b = np.asarray(ln_b, f32)
    w1 = np.asarray(w1, f32)
    b1 = np.asarray(b1, f32)
    w2 = np.asarray(w2, f32)
    b2 = np.asarray(b2, f32)

    pe_taps, dve_taps = _taps()

    # fold LN affine into w1/b1
    w1g = (ln_g[:, None] * w1).astype(f32)                  # [256, 1024]
    b1e = (ln_b @ w1 + b1).astype(f32)                      # [1024]

    # w1sb[c, ct*1024 + f*128 + j] = w1g[ct*128 + c, f*128 + j]
    w1sb = np.ascontiguousarray(
        w1g.reshape(2, 128, 8, 128).transpose(1, 0, 2, 3).reshape(128, 2048)
    )
    # w2sb[h, f*256 + ct*128 + co] = w2[f*128 + h, ct*128 + co]
    w2sb = np.ascontiguousarray(
        w2.reshape(8, 128, 2, 128).transpose(1, 0, 2, 3).reshape(128, 2048)
    )
    # diagonal conv matrices for PE taps
    dgsb = np.zeros((128, NCT * N_PE_TAPS * 128), f32)
    idx = np.arange(128)
    for ct in range(NCT):
        for i, (_, di, dj) in enumerate(pe_taps):
            dgsb[idx, (ct * N_PE_TAPS + i) * 128 + idx] = conv_w[ct * 128 + idx, 0, di, dj]
    ones128 = np.ones((128, 128), f32)

    NC_CONST = 13 + NCT * N_DVE_TAPS
    cst = np.zeros((128, NC_CONST), f32)
    cst[:, 0] = conv_b[:128]
    cst[:, 1] = conv_b[128:]
    cst[:, 2:10] = b1e.reshape(8, 128).T
    cst[:, 10] = b2[:128]
    cst[:, 11] = b2[128:]
    cst[:, 12] = EPS
    for ct in range(NCT):
        for j, (_, di, dj) in enumerate(dve_taps):
            cst[:, 13 + ct * N_DVE_TAPS + j] = conv_w[ct * 128 + idx, 0, di, dj]

    # padded input grids
    xg = np.zeros((B, DIM, G, G), f32)
    xg[:, :, 3:59, 3:59] = x
    xg = xg.reshape(B, NCT, 128, G * G)
    xpad = np.zeros((B, NCT, 128, XPL), f32)
    xpad[:, :, :, 3:3 + G * G] = xg

    static = dict(w1sb=w1sb, w2sb=w2sb, dgsb=dgsb, ones128=ones128, cstf=cst)
    xpads = [np.ascontiguousarray(xpad[c * BS:(c + 1) * BS]) for c in range(NCORES)]
    return static, xpads


def kernel(**inputs) -> np.ndarray:
    from concourse import bass_utils

    if "nc" not in _CACHE:
        _CACHE["nc"] = _build_program()
    nc = _CACHE["nc"]

    static, xpads = _host_prep(**inputs)
    in_maps = [dict(static, xpad=xpads[c]) for c in range(NCORES)]
    res = bass_utils.run_bass_kernel_spmd(nc, in_maps, core_ids=list(range(NCORES)))

    out = np.empty((B, DIM, H, W), np.float32)
    for c in range(NCORES):
        yo = res.results[c]["yout"].reshape(BS, NCT, 128, H, W)
        for b in range(BS):
            out[c * BS + b, :128] = yo[b, 0]
            out[c * BS + b, 128:] = yo[b, 1]
    return out
